# revision 57
# baseline (speedup 1.0000x reference)
"""Multi-head causal attention (B=2, S=4096, D=512, H=8) on 8 NeuronCores.

Sharding: batch x head-pair. Core c handles batch b = c//4 and heads
{2*(c%4), 2*(c%4)+1}. Each core computes its 2 heads' projections, causal
flash attention, and a partial out-projection (its heads' rank-128 slice of
W_o). Partials of the 4 cores sharing a batch are summed on the host during
the gather (tensor-parallel all-reduce); b_o is added on the host too.

Device design:
  - scores computed transposed: S.T [k, q] tiles so PV needs no transposes;
    per-q row-sums come from an ones-column appended to V (PV matmul M=65)
  - softmax without a running max; for blocks j>=1 the exp is biased by -3
    (exp(s/8 - 3)) so probabilities fit fp8e4 range; the bias cancels in the
    normalization. Block j=0 (first 512 q) keeps a bf16 path since its early
    rows can have tiny row maxima that would flush to zero in fp8.
  - exp is split per (k-tile, head) across two engines: roughly half the
    head-halves use the Scalar ACT table exp, the rest compute exp on
    VectorE as a Schraudolph-style affine in fp8-bitpattern space (uint8
    saturating convert of s*A8+B8 == e4m3 bits of exp(s/8-3), within ~6%);
    each tile's two exps run concurrently on two engines, and the per-head
    1-bank st slots (psA bufs=4) give four parallel exp->QK recycle chains
  - PV for j>=1 runs as fp8 DoubleRow matmuls over k-tile PAIRS:
    lhsT = vho8[128, 2, 65] (stride-160 pair slices), rhs = pt8[128, 2, 512]
  - V is projected directly into transposed [kpos, d] layout (lhsT = the
    x chunk, rhs = W_v slice), so no PE transposes / identity matrix
  - causal masking decomposed: the triangular boundary is only a [128, 128]
    subtile (one shared local-triangle multiply on VectorE); fully-masked
    128-col stripes are memset to 0 on GpSimd; fully-masked column PAIRS
    are skipped in QK/exp/PV
  - the normalization + out-projection of each block is emitted DEFERRED,
    one stage per subsequent item, so its cross-engine chain (scalar ones-
    row copy -> vector recip -> gpsimd broadcast -> vector mul -> PE
    out-proj) never blocks the next block's per-tile work in any queue
  - projection work is spread across the item loop (one part per QK)
"""

import numpy as np
import ml_dtypes

import concourse.bass as bass
import concourse.bacc as bacc
import concourse.mybir as mybir
import concourse.tile as tile
from concourse.bass_utils import run_bass_kernel_spmd

D = 512
BSHIFT = 3.0  # exp bias for fp8 path: pt = exp(s/8 - BSHIFT)
LOG2E = float(np.log2(np.e))
A8 = 0.125 * LOG2E * 8.0
B8 = 8.0 * (7.0 - BSHIFT * LOG2E - 0.0436)

f32 = mybir.dt.float32
bf16 = mybir.dt.bfloat16
fp8 = mybir.dt.float8e4
u8 = mybir.dt.uint8
ts = bass.ts
Act = mybir.ActivationFunctionType
Alu = mybir.AluOpType
DR = mybir.MatmulPerfMode.DoubleRow


def build(S=4096):
    NQB = S // 512  # q-blocks / s-blocks / k-groups

    nc = bacc.Bacc("TRN2", target_bir_lowering=False, debug=False, num_devices=8)

    qT_d = nc.dram_tensor("qT", [D, S], bf16, kind="ExternalInput").ap()
    kT_d = nc.dram_tensor("kT", [D, S], bf16, kind="ExternalInput").ap()
    vT_d = nc.dram_tensor("vT", [D, S], bf16, kind="ExternalInput").ap()
    wqT_d = nc.dram_tensor("wqT", [128, D], bf16, kind="ExternalInput").ap()
    wkT_d = nc.dram_tensor("wkT", [128, D], bf16, kind="ExternalInput").ap()
    wvT_d = nc.dram_tensor("wvT", [128, D], bf16, kind="ExternalInput").ap()
    woT_d = nc.dram_tensor("woT", [128, D], bf16, kind="ExternalInput").ap()
    masks_d = nc.dram_tensor("masks", [128, 128], bf16, kind="ExternalInput").ap()
    masks8_d = nc.dram_tensor("masks8", [128, 128], fp8, kind="ExternalInput").ap()
    outT_d = nc.dram_tensor("outT", [D, S], f32, kind="ExternalOutput").ap()

    with tile.TileContext(nc) as tc:
        with (
            tc.tile_pool(name="const", bufs=1) as pc,
            tc.tile_pool(name="persist", bufs=1) as pp,
            tc.tile_pool(name="chunk", bufs=10) as pch,
            tc.tile_pool(name="pt", bufs=4) as ppt,
            tc.tile_pool(name="pt8", bufs=8) as ppt8,
            tc.tile_pool(name="small", bufs=3) as psm,
            tc.tile_pool(name="ostage", bufs=2) as pos,
            tc.tile_pool(name="psP", bufs=2, space="PSUM") as psP,
            tc.tile_pool(name="psA", bufs=2, space="PSUM") as psA,
            tc.tile_pool(name="psC", bufs=2, space="PSUM") as psC,
        ):
            tri = pc.tile([128, 128], bf16, tag="tri")
            tri8 = pc.tile([128, 128], fp8, tag="tri8")
            wq = pc.tile([128, 4, 128], bf16, tag="wq")
            wk = pc.tile([128, 4, 128], bf16, tag="wk")
            wv = pc.tile([128, 4, 128], bf16, tag="wv")
            wo = pc.tile([128, D], bf16, tag="wo")
            nbias = pc.tile([128, 1], f32, tag="nbias")
            nc.gpsimd.memset(nbias[:], -BSHIFT)
            nc.sync.dma_start(wk[:], wkT_d.rearrange("p (e m) -> p e m", e=4))
            nc.sync.dma_start(wq[:], wqT_d.rearrange("p (e m) -> p e m", e=4))
            nc.sync.dma_start(wv[:], wvT_d.rearrange("p (e m) -> p e m", e=4))

            def emit_consts():
                nc.sync.dma_start(tri[:], masks_d)
                nc.sync.dma_start(tri8[:], masks8_d)
                nc.sync.dma_start(wo[:], woT_d)

            khT = [pp.tile([128, 512], bf16, tag=f"khT{g}", name=f"khT{g}") for g in range(NQB)]
            qhT = [pp.tile([128, 512], bf16, tag=f"qhT{g}", name=f"qhT{g}") for g in range(NQB)]
            ctxT = [pp.tile([128, 512], bf16, tag=f"ctxT{g}", name=f"ctxT{g}") for g in range(NQB)]
            # [128 kpos, 4 u, 2 heads, 80]: fp8 V with ones col 64; pair slice
            # [:, u:u+2, h, 0:65] has dim1 stride 160 (%16==0 for dual-fp8 LDW)
            vho8 = [pp.tile([128, 4, 2, 80], fp8, tag=f"vho8_{g}", name=f"vho8_{g}") for g in range(NQB)]
            vho16 = pp.tile([128, 4, 2, 65], bf16, tag="vho16", name="vho16")
            nc.gpsimd.memset(vho16[:, :, :, 64:65], 1.0)
            for g in range(NQB):
                nc.gpsimd.memset(vho8[g][:, :, :, 64:65], 1.0)

            # ---------------------------------------------------------------

            def dma_proj(j, src_d):
                """Prefetch all 4 input chunks of a projection in ONE strided
                DMA ([512, 512] dram block -> [128, 4, 512] tile) so neither
                DMA latency nor Sync-queue issue cost gates the matmuls."""
                ch = pch.tile([128, 4, 512], bf16, tag="chunk", name="ch")
                nc.sync.dma_start(
                    ch[:], src_d[:, ts(j, 512)].rearrange("(e p) c -> p e c", e=4)
                )
                return ch

            def emit_proj_kq(j, ch, w, dst):
                slot = psP.tile([128, 512], f32, tag="pp", name="pp")
                for e in range(4):
                    nc.tensor.matmul(
                        slot[:], w[:, e, :], ch[:, e, :], start=(e == 0), stop=(e == 3)
                    )
                nc.scalar.copy(dst[j][:], slot[:])

            def emit_proj_v(j, ch, us):
                # v projected straight into [kpos, d2] layout for subchunks us
                slot = psP.tile([128, 512], f32, tag="pp", name="slotv")
                for uu in us:
                    for e in range(4):
                        nc.tensor.matmul(
                            slot[:, ts(uu, 128)],
                            ch[:, e, ts(uu, 128)],
                            wv[:, e, :],
                            start=(e == 0),
                            stop=(e == 3),
                        )
                    src = slot[:, ts(uu, 128)].rearrange("p (h d) -> p h d", h=2)
                    nc.vector.tensor_copy(vho8[j][:, uu, :, 0:64], src)
                    if j == 0:
                        nc.vector.tensor_copy(vho16[:, uu, :, 0:64], src)

            pending_parts = []

            def queue_proj(j):
                chk = dma_proj(j, kT_d)
                chq = dma_proj(j, qT_d)
                chv = dma_proj(j, vT_d)
                pending_parts.append(lambda: emit_proj_kq(j, chk, wk, khT))
                pending_parts.append(lambda: emit_proj_kq(j, chq, wq, qhT))
                pending_parts.append(lambda: emit_proj_v(j, chv, (0, 1)))
                pending_parts.append(lambda: emit_proj_v(j, chv, (2, 3)))

            def emit_proj(j):
                emit_proj_kq(j, dma_proj(j, kT_d), wk, khT)
                emit_proj_kq(j, dma_proj(j, qT_d), wq, qhT)
                chv = dma_proj(j, vT_d)
                emit_proj_v(j, chv, (0, 1))
                emit_proj_v(j, chv, (2, 3))

            deferred = []

            ob_tiles = {}

            def emit_outproj_ot(j, ot):
                op = psP.tile([128, 512], f32, tag="pp", name="op")
                nc.tensor.matmul(
                    op[:], wo[:, ts(ot, 128)], ctxT[j][:], start=True, stop=True
                )
                if ot == 0:
                    ob_tiles[j] = pos.tile([128, 4, 512], f32, tag="ob", name="ob")
                ob = ob_tiles[j]
                if ot % 2 == 0:
                    nc.vector.tensor_copy(ob[:, ot, :], op[:])
                else:
                    nc.scalar.copy(ob[:, ot, :], op[:])
                # per-ot DMA, dispatched from the (idle) gpsimd queue right
                # after its copy: keeps outT traffic off the Sync queue AND
                # avoids one huge serialized transfer blocking the gpsimd
                # queue (whose next op is the following block's broadcast)
                nc.gpsimd.dma_start(outT_d[ts(ot, 128), ts(j, 512)], ob[:, ot, :])
                if ot == 3:
                    ob_tiles.pop(j)

            ctx_tiles = {}
            st_tiles = {}
            st_pairs = {}
            pt8_tiles = {}
            norm_state = {}

            def emit_norm(j):
                """Queue the normalization + out-projection stages for block j;
                they are drained one per subsequent item so they never block
                the next block's per-tile work."""
                ctxs = [ctx_tiles.pop((j, h)) for h in range(2)]

                # stage 1 immediately: ones-row copy + reciprocal
                lrow = psm.tile([1, 2, 512], f32, tag="lrow", name="lrow", bufs=2)
                for h in range(2):
                    nc.scalar.copy(lrow[:, h, :], ctxs[h][64:65, :])
                r = psm.tile([1, 2, 512], f32, tag="r", name="r", bufs=2)
                nc.vector.reciprocal_approx_fast(
                    r[:].rearrange("p a b -> p (a b)"),
                    lrow[:].rearrange("p a b -> p (a b)"),
                )
                norm_state[j] = r

                def stage2():
                    r = norm_state.pop(j)
                    rbc = psm.tile([64, 2, 512], f32, tag="rbc", name="rbc", bufs=2)
                    nc.gpsimd.partition_broadcast(
                        rbc[:].rearrange("p a b -> p (a b)"),
                        r[:].rearrange("p a b -> p (a b)"),
                    )
                    for h in range(2):
                        nc.vector.tensor_mul(
                            ctxT[j][64 * h : 64 * h + 64, :],
                            ctxs[h][0:64, :],
                            rbc[:, h, :],
                        )

                deferred.append(stage2)
                for ot in range(4):
                    deferred.append(lambda ot=ot: emit_outproj_ot(j, ot))

            def c0_of(j, t):
                # pair-granular masked-column start (PV + memset base)
                u = t - 4 * j
                if j >= 1 and u >= 2:
                    return 256
                return 0

            def c0_qk(j, t):
                # tile-granular start for QK + exp: columns below 128*u are
                # fully masked; the gpless memset zeroes [c0_of, 128u) of pt
                # so PV still reads valid zeros there
                u = t - 4 * j
                if u >= 1:
                    return 128 * u
                return 0

            def emit_qk(i):
                j, t = items[i]
                if t == 0 and j + 2 < NQB and j + 2 >= 2:
                    queue_proj(j + 2)
                # per-(pair, head) 2-bank st slots: both tiles of a k-tile
                # pair share one slab per head, so off-diagonal pairs can run
                # ONE fused exp per head over [128, 2, 512]
                p, tt = divmod(t, 2)
                if tt == 0:
                    st_pairs[p] = [
                        psA.tile([128, 2, 512], f32, tag="st", name=f"st{h}")
                        for h in range(2)
                    ]
                sts = st_pairs[p] if tt == 0 else st_pairs.pop(p)
                c0 = c0_qk(j, t)
                for h in range(2):
                    nc.tensor.matmul(
                        sts[h][:, tt, c0:512],
                        khT[t // 4][64 * h : 64 * h + 64, ts(t % 4, 128)],
                        qhT[j][64 * h : 64 * h + 64, c0:512],
                        start=True, stop=True, tile_position=(64 * h, 0),
                    )
                st_tiles[i] = (sts, c0)
                # proj work AFTER the QK pair: it fills the PE's exp-wait
                # time instead of sitting between the freed st slot and the
                # next QK in the queue
                if pending_parts:
                    pending_parts.pop(0)()

            def emit_mask(sl, j, t, c0, trit):
                """Causal masking for diagonal tile (j, t), one head: memset
                the fully masked 128-col stripes, multiply the local triangle.
                sl(a, b) -> the [128, b-a] AP of this head's columns a:b."""
                u = t - 4 * j
                if 128 * u > c0:
                    nc.vector.memset(sl(c0, 128 * u), 0.0)
                nc.vector.tensor_mul(
                    sl(128 * u, 128 * u + 128),
                    sl(128 * u, 128 * u + 128),
                    trit[:],
                )

            def use_act(i, h):
                # chains (2i+h)%4 in {0, 3} -> ACT, {1, 2} -> DVE, plus a few
                # DVE-chain tiles shifted to ACT for engine balance
                c = (2 * i + h) % 4
                if c in (0, 3):
                    return True
                return i % 8 == 1 and h == 0

            def emit_pv_j0(i):
                """bf16 per-tile path for block j=0 (t = 0..3, all diagonal)."""
                j, t = items[i]
                sts, cq = st_tiles.pop(i)
                if t == 0:
                    ctx_tiles[(j, 0)] = psC.tile([65, 512], f32, tag="ctx", name="ctx0")
                    ctx_tiles[(j, 1)] = psC.tile([65, 512], f32, tag="ctx", name="ctx1")
                for h in range(2):
                    pt = ppt.tile([128, 512], bf16, tag="pt", name="pt")
                    nc.scalar.activation(
                        pt[:, cq:512], sts[h][:, t % 2, cq:512], Act.Exp, scale=0.125
                    )
                    emit_mask(lambda a, b: pt[:, a:b], 0, t, 0, tri)
                    nc.tensor.matmul(
                        ctx_tiles[(j, h)][:],
                        vho16[:, t, h, :],
                        pt[:],
                        start=(t == 0),
                        stop=(t == 3),
                    )
                if t == 3:
                    emit_norm(j)

            def emit_pv(i):
                """fp8 DoubleRow pair path for blocks j>=1."""
                j, t = items[i]
                sts, cq = st_tiles.pop(i)
                c0 = c0_of(j, t)
                p, tt = divmod(t, 2)
                if tt == 0:
                    for h in range(2):
                        pt8_tiles[(p, h)] = ppt8.tile(
                            [128, 2, 512], fp8, tag="pt8", name="pt8"
                        )
                if t == 0:
                    ctx_tiles[(j, 0)] = psC.tile([65, 512], f32, tag="ctx", name="ctx0")
                    ctx_tiles[(j, 1)] = psC.tile([65, 512], f32, tag="ctx", name="ctx1")
                diag = t - 4 * j >= 0
                for h in range(2):
                    pt8 = pt8_tiles[(p, h)]
                    if diag:
                        # per-tile exp keeps the tile-granular column trim
                        if use_act(i, h):
                            nc.scalar.activation(
                                pt8[:, tt, cq:512], sts[h][:, tt, cq:512],
                                Act.Exp, scale=0.125, bias=nbias[:],
                            )
                        else:
                            nc.vector.tensor_scalar(
                                pt8[:, tt, cq:512].bitcast(u8),
                                sts[h][:, tt, cq:512],
                                A8, B8, Alu.mult, Alu.add,
                            )
                        emit_mask(lambda a, b: pt8[:, tt, a:b], j, t, c0, tri8)
                    elif tt == 1:
                        # off-diagonal pair: one fused exp per head over the
                        # whole [128, 2, 512] slab - half the instruction
                        # overhead, true dependency on both QKs
                        if use_act(p, h):
                            nc.scalar.activation(
                                pt8[:], sts[h][:], Act.Exp,
                                scale=0.125, bias=nbias[:],
                            )
                        else:
                            nc.vector.tensor_scalar(
                                pt8[:].bitcast(u8), sts[h][:],
                                A8, B8, Alu.mult, Alu.add,
                            )
                if tt == 1:
                    g, up = divmod(2 * p, 4)
                    for h in range(2):
                        pt8 = pt8_tiles.pop((p, h))
                        nc.tensor.matmul(
                            ctx_tiles[(j, h)][:, c0:512],
                            vho8[g][:, up : up + 2, h, 0:65],
                            pt8[:, :, c0:512],
                            start=(t == 1),
                            stop=(t == 4 * j + 3),
                            perf_mode=DR,
                        )
                    if t == 4 * j + 3:
                        emit_norm(j)

            # ---------------------------------------------------------------
            items = [(j, t) for j in range(NQB) for t in range(4 * j + 4)]
            emit_proj(0)
            emit_qk(0)
            if len(items) > 1:
                emit_qk(1)
            if NQB > 1:
                emit_proj(1)
            emit_consts()
            for i in range(len(items)):
                if i + 2 < len(items):
                    emit_qk(i + 2)
                if deferred:
                    deferred.pop(0)()
                if items[i][0] == 0:
                    emit_pv_j0(i)
                else:
                    emit_pv(i)
            while deferred:
                deferred.pop(0)()

    nc.compile()
    return nc


def make_in_maps(q, k, v, W_q, W_k, W_v, W_o, b_o, S=4096):
    B = q.shape[0]
    q = np.asarray(q, dtype=np.float32)
    k = np.asarray(k, dtype=np.float32)
    v = np.asarray(v, dtype=np.float32)
    W_q = np.asarray(W_q, dtype=np.float32)
    W_k = np.asarray(W_k, dtype=np.float32)
    W_v = np.asarray(W_v, dtype=np.float32)
    W_o = np.asarray(W_o, dtype=np.float32)
    bf = ml_dtypes.bfloat16
    e4 = ml_dtypes.float8_e4m3

    qT = [np.ascontiguousarray(q[b].T).astype(bf) for b in range(B)]
    kT = [np.ascontiguousarray(k[b].T).astype(bf) for b in range(B)]
    vT = [np.ascontiguousarray(v[b].T).astype(bf) for b in range(B)]

    kk = np.arange(128)[:, None]
    qq = np.arange(128)[None, :]
    trif = (kk <= qq).astype(np.float32)  # [128, 128] local triangle

    in_maps = []
    for c in range(8):
        b, p = divmod(c, 4)
        rows = slice(128 * p, 128 * p + 128)

        def wtile(W):
            wT = W[rows].T.reshape(4, 128, 128).transpose(1, 0, 2)
            return np.ascontiguousarray(wT).astype(bf).reshape(128, 512)
        in_maps.append(
            {
                "qT": qT[b],
                "kT": kT[b],
                "vT": vT[b],
                "wqT": wtile(W_q),
                "wkT": wtile(W_k),
                "wvT": wtile(W_v),
                "woT": np.ascontiguousarray(W_o[:, rows].T).astype(bf),
                "masks": trif.astype(bf),
                "masks8": trif.astype(e4),
            }
        )
    return in_maps


def gather(results, b_o=None, S=4096):
    outT = [r["outT"] for r in results]
    out0 = (outT[0] + outT[1] + outT[2] + outT[3]).T
    out1 = (outT[4] + outT[5] + outT[6] + outT[7]).T
    out = np.stack([out0, out1]).astype(np.float32)
    if b_o is not None:
        out += np.asarray(b_o, dtype=np.float32)
    return out


_nc_cache = {}


def get_nc(S=4096):
    if S not in _nc_cache:
        _nc_cache[S] = build(S)
    return _nc_cache[S]


def kernel(q, k, v, W_q, W_k, W_v, W_o, b_o):
    nc = get_nc(4096)
    in_maps = make_in_maps(q, k, v, W_q, W_k, W_v, W_o, b_o, S=4096)
    res = run_bass_kernel_spmd(nc, in_maps, core_ids=list(range(8)))
    return gather(res.results, b_o=b_o)


# revision 58
# speedup vs baseline: 1.1366x; 1.1366x over previous
"""Multi-head causal attention (B=2, S=4096, D=512, H=8) on 8 NeuronCores.

Sharding: batch x head-pair. Core c handles batch b = c//4 and heads
{2*(c%4), 2*(c%4)+1}. Each core computes its 2 heads' projections, causal
flash attention, and a partial out-projection (its heads' rank-128 slice of
W_o). Partials of the 4 cores sharing a batch are summed on the host during
the gather (tensor-parallel all-reduce); b_o is added on the host too.

Device design:
  - scores computed transposed: S.T [k, q] tiles so PV needs no transposes;
    per-q row-sums come from an ones-column appended to V (PV matmul M=65)
  - softmax without a running max; for blocks j>=1 the exp is biased by -3
    (exp(s/8 - 3)) so probabilities fit fp8e4 range; the bias cancels in the
    normalization. Block j=0 (first 512 q) keeps a bf16 path since its early
    rows can have tiny row maxima that would flush to zero in fp8.
  - exp is split per (k-tile, head) across two engines: roughly half the
    head-halves use the Scalar ACT table exp, the rest compute exp on
    VectorE as a Schraudolph-style affine in fp8-bitpattern space (uint8
    saturating convert of s*A8+B8 == e4m3 bits of exp(s/8-3), within ~6%);
    each tile's two exps run concurrently on two engines, and the per-head
    1-bank st slots (psA bufs=4) give four parallel exp->QK recycle chains
  - PV for j>=1 runs as fp8 DoubleRow matmuls over k-tile PAIRS:
    lhsT = vho8[128, 2, 65] (stride-160 pair slices), rhs = pt8[128, 2, 512]
  - V is projected directly into transposed [kpos, d] layout (lhsT = the
    x chunk, rhs = W_v slice), so no PE transposes / identity matrix
  - causal masking decomposed: the triangular boundary is only a [128, 128]
    subtile (one shared local-triangle multiply on VectorE); fully-masked
    128-col stripes are memset to 0 on GpSimd; fully-masked column PAIRS
    are skipped in QK/exp/PV
  - the normalization + out-projection of each block is emitted DEFERRED,
    one stage per subsequent item, so its cross-engine chain (scalar ones-
    row copy -> vector recip -> gpsimd broadcast -> vector mul -> PE
    out-proj) never blocks the next block's per-tile work in any queue
  - projection work is spread across the item loop (one part per QK)
"""

import numpy as np
import ml_dtypes

import concourse.bass as bass
import concourse.bacc as bacc
import concourse.mybir as mybir
import concourse.tile as tile
from concourse.bass_utils import run_bass_kernel_spmd

D = 512
BSHIFT = 3.0  # exp bias for fp8 path: pt = exp(s/8 - BSHIFT)
LOG2E = float(np.log2(np.e))
A8 = 0.125 * LOG2E * 8.0
B8 = 8.0 * (7.0 - BSHIFT * LOG2E - 0.0436)

f32 = mybir.dt.float32
bf16 = mybir.dt.bfloat16
fp8 = mybir.dt.float8e4
u8 = mybir.dt.uint8
ts = bass.ts
Act = mybir.ActivationFunctionType
Alu = mybir.AluOpType
DR = mybir.MatmulPerfMode.DoubleRow


def build(S=4096):
    NQB = S // 512  # q-blocks / s-blocks / k-groups

    nc = bacc.Bacc("TRN2", target_bir_lowering=False, debug=False, num_devices=8)

    qT_d = nc.dram_tensor("qT", [D, S], bf16, kind="ExternalInput").ap()
    kT_d = nc.dram_tensor("kT", [D, S], bf16, kind="ExternalInput").ap()
    vT_d = nc.dram_tensor("vT", [D, S], bf16, kind="ExternalInput").ap()
    wqT_d = nc.dram_tensor("wqT", [128, D], bf16, kind="ExternalInput").ap()
    wkT_d = nc.dram_tensor("wkT", [128, D], bf16, kind="ExternalInput").ap()
    wvT_d = nc.dram_tensor("wvT", [128, D], bf16, kind="ExternalInput").ap()
    woT_d = nc.dram_tensor("woT", [128, D], bf16, kind="ExternalInput").ap()
    masks_d = nc.dram_tensor("masks", [128, 128], bf16, kind="ExternalInput").ap()
    masks8_d = nc.dram_tensor("masks8", [128, 128], fp8, kind="ExternalInput").ap()
    outT_d = nc.dram_tensor("outT", [D, S], f32, kind="ExternalOutput").ap()

    with tile.TileContext(nc) as tc:
        with (
            tc.tile_pool(name="const", bufs=1) as pc,
            tc.tile_pool(name="persist", bufs=1) as pp,
            tc.tile_pool(name="chunk", bufs=10) as pch,
            tc.tile_pool(name="pt", bufs=4) as ppt,
            tc.tile_pool(name="pt8", bufs=8) as ppt8,
            tc.tile_pool(name="small", bufs=3) as psm,
            tc.tile_pool(name="ostage", bufs=2) as pos,
            tc.tile_pool(name="psP", bufs=2, space="PSUM") as psP,
            tc.tile_pool(name="psA", bufs=4, space="PSUM") as psA,
            tc.tile_pool(name="psC", bufs=2, space="PSUM") as psC,
        ):
            tri = pc.tile([128, 128], bf16, tag="tri")
            tri8 = pc.tile([128, 128], fp8, tag="tri8")
            wq = pc.tile([128, 4, 128], bf16, tag="wq")
            wk = pc.tile([128, 4, 128], bf16, tag="wk")
            wv = pc.tile([128, 4, 128], bf16, tag="wv")
            wo = pc.tile([128, D], bf16, tag="wo")
            nbias = pc.tile([128, 1], f32, tag="nbias")
            nc.gpsimd.memset(nbias[:], -BSHIFT)
            nc.sync.dma_start(wk[:], wkT_d.rearrange("p (e m) -> p e m", e=4))
            nc.sync.dma_start(wq[:], wqT_d.rearrange("p (e m) -> p e m", e=4))
            nc.sync.dma_start(wv[:], wvT_d.rearrange("p (e m) -> p e m", e=4))

            def emit_consts():
                nc.sync.dma_start(tri[:], masks_d)
                nc.sync.dma_start(tri8[:], masks8_d)
                nc.sync.dma_start(wo[:], woT_d)

            khT = [pp.tile([128, 512], bf16, tag=f"khT{g}", name=f"khT{g}") for g in range(NQB)]
            qhT = [pp.tile([128, 512], bf16, tag=f"qhT{g}", name=f"qhT{g}") for g in range(NQB)]
            ctxT = [pp.tile([128, 512], bf16, tag=f"ctxT{g}", name=f"ctxT{g}") for g in range(NQB)]
            # [128 kpos, 4 u, 2 heads, 80]: fp8 V with ones col 64; pair slice
            # [:, u:u+2, h, 0:65] has dim1 stride 160 (%16==0 for dual-fp8 LDW)
            vho8 = [pp.tile([128, 4, 2, 80], fp8, tag=f"vho8_{g}", name=f"vho8_{g}") for g in range(NQB)]
            vho16 = pp.tile([128, 4, 2, 65], bf16, tag="vho16", name="vho16")
            nc.gpsimd.memset(vho16[:, :, :, 64:65], 1.0)
            for g in range(NQB):
                nc.gpsimd.memset(vho8[g][:, :, :, 64:65], 1.0)

            # ---------------------------------------------------------------

            def dma_proj(j, src_d):
                """Prefetch all 4 input chunks of a projection in ONE strided
                DMA ([512, 512] dram block -> [128, 4, 512] tile) so neither
                DMA latency nor Sync-queue issue cost gates the matmuls."""
                ch = pch.tile([128, 4, 512], bf16, tag="chunk", name="ch")
                nc.sync.dma_start(
                    ch[:], src_d[:, ts(j, 512)].rearrange("(e p) c -> p e c", e=4)
                )
                return ch

            def emit_proj_kq(j, ch, w, dst):
                slot = psP.tile([128, 512], f32, tag="pp", name="pp")
                for e in range(4):
                    nc.tensor.matmul(
                        slot[:], w[:, e, :], ch[:, e, :], start=(e == 0), stop=(e == 3)
                    )
                nc.scalar.copy(dst[j][:], slot[:])

            def emit_proj_v(j, ch, us):
                # v projected straight into [kpos, d2] layout for subchunks us
                slot = psP.tile([128, 512], f32, tag="pp", name="slotv")
                for uu in us:
                    for e in range(4):
                        nc.tensor.matmul(
                            slot[:, ts(uu, 128)],
                            ch[:, e, ts(uu, 128)],
                            wv[:, e, :],
                            start=(e == 0),
                            stop=(e == 3),
                        )
                    src = slot[:, ts(uu, 128)].rearrange("p (h d) -> p h d", h=2)
                    nc.vector.tensor_copy(vho8[j][:, uu, :, 0:64], src)
                    if j == 0:
                        nc.vector.tensor_copy(vho16[:, uu, :, 0:64], src)

            pending_parts = []

            def queue_proj(j):
                chk = dma_proj(j, kT_d)
                chq = dma_proj(j, qT_d)
                chv = dma_proj(j, vT_d)
                pending_parts.append(lambda: emit_proj_kq(j, chk, wk, khT))
                pending_parts.append(lambda: emit_proj_kq(j, chq, wq, qhT))
                pending_parts.append(lambda: emit_proj_v(j, chv, (0, 1)))
                pending_parts.append(lambda: emit_proj_v(j, chv, (2, 3)))

            def emit_proj(j):
                emit_proj_kq(j, dma_proj(j, kT_d), wk, khT)
                emit_proj_kq(j, dma_proj(j, qT_d), wq, qhT)
                chv = dma_proj(j, vT_d)
                emit_proj_v(j, chv, (0, 1))
                emit_proj_v(j, chv, (2, 3))

            deferred = []

            ob_tiles = {}

            def emit_outproj_ot(j, ot):
                op = psP.tile([128, 512], f32, tag="pp", name="op")
                nc.tensor.matmul(
                    op[:], wo[:, ts(ot, 128)], ctxT[j][:], start=True, stop=True
                )
                if ot == 0:
                    ob_tiles[j] = pos.tile([128, 4, 512], f32, tag="ob", name="ob")
                ob = ob_tiles[j]
                if ot % 2 == 0:
                    nc.vector.tensor_copy(ob[:, ot, :], op[:])
                else:
                    nc.scalar.copy(ob[:, ot, :], op[:])
                # per-ot DMA, dispatched from the (idle) gpsimd queue right
                # after its copy: keeps outT traffic off the Sync queue AND
                # avoids one huge serialized transfer blocking the gpsimd
                # queue (whose next op is the following block's broadcast)
                nc.gpsimd.dma_start(outT_d[ts(ot, 128), ts(j, 512)], ob[:, ot, :])
                if ot == 3:
                    ob_tiles.pop(j)

            ctx_tiles = {}
            st_tiles = {}
            pt8_tiles = {}
            norm_state = {}

            def emit_norm(j):
                """Queue the normalization + out-projection stages for block j;
                they are drained one per subsequent item so they never block
                the next block's per-tile work."""
                ctxs = [ctx_tiles.pop((j, h)) for h in range(2)]

                # stage 1 immediately: ones-row copy + reciprocal
                lrow = psm.tile([1, 2, 512], f32, tag="lrow", name="lrow", bufs=2)
                for h in range(2):
                    nc.scalar.copy(lrow[:, h, :], ctxs[h][64:65, :])
                r = psm.tile([1, 2, 512], f32, tag="r", name="r", bufs=2)
                nc.vector.reciprocal_approx_fast(
                    r[:].rearrange("p a b -> p (a b)"),
                    lrow[:].rearrange("p a b -> p (a b)"),
                )
                norm_state[j] = r

                def stage2():
                    r = norm_state.pop(j)
                    rbc = psm.tile([64, 2, 512], f32, tag="rbc", name="rbc", bufs=2)
                    nc.gpsimd.partition_broadcast(
                        rbc[:].rearrange("p a b -> p (a b)"),
                        r[:].rearrange("p a b -> p (a b)"),
                    )
                    for h in range(2):
                        nc.vector.tensor_mul(
                            ctxT[j][64 * h : 64 * h + 64, :],
                            ctxs[h][0:64, :],
                            rbc[:, h, :],
                        )

                deferred.append(stage2)
                for ot in range(4):
                    deferred.append(lambda ot=ot: emit_outproj_ot(j, ot))

            def c0_of(j, t):
                # pair-granular masked-column start (PV + memset base)
                u = t - 4 * j
                if j >= 1 and u >= 2:
                    return 256
                return 0

            def c0_qk(j, t):
                # tile-granular start for QK + exp: columns below 128*u are
                # fully masked; the gpless memset zeroes [c0_of, 128u) of pt
                # so PV still reads valid zeros there
                u = t - 4 * j
                if u >= 1:
                    return 128 * u
                return 0

            def emit_qk(i):
                j, t = items[i]
                if t == 0 and j + 2 < NQB and j + 2 >= 2:
                    queue_proj(j + 2)
                # per-head 1-bank st slots; 5 slots so the second head's
                # slot is almost always free and the QK pair launches
                # concurrently on the PE
                sts = [
                    psA.tile([128, 512], f32, tag="st", name=f"st{h}")
                    for h in range(2)
                ]
                c0 = c0_qk(j, t)
                for h in range(2):
                    nc.tensor.matmul(
                        sts[h][:, c0:512],
                        khT[t // 4][64 * h : 64 * h + 64, ts(t % 4, 128)],
                        qhT[j][64 * h : 64 * h + 64, c0:512],
                        start=True, stop=True, tile_position=(64 * h, 0),
                    )
                st_tiles[i] = (sts, c0)
                # proj work AFTER the QK pair: it fills the PE's exp-wait
                # time instead of sitting between the freed st slot and the
                # next QK in the queue
                if pending_parts:
                    pending_parts.pop(0)()

            def emit_mask(sl, j, t, c0, trit):
                """Causal masking for diagonal tile (j, t), one head: memset
                the fully masked 128-col stripes, multiply the local triangle.
                sl(a, b) -> the [128, b-a] AP of this head's columns a:b."""
                u = t - 4 * j
                if 128 * u > c0:
                    nc.vector.memset(sl(c0, 128 * u), 0.0)
                nc.vector.tensor_mul(
                    sl(128 * u, 128 * u + 128),
                    sl(128 * u, 128 * u + 128),
                    trit[:],
                )

            def use_act(i, h):
                # chains (2i+h)%4 in {0, 3} -> ACT, {1, 2} -> DVE, plus a few
                # DVE-chain tiles shifted to ACT for engine balance
                c = (2 * i + h) % 4
                if c in (0, 3):
                    return True
                return i % 8 == 1 and h == 0

            def emit_pv_j0(i):
                """bf16 per-tile path for block j=0 (t = 0..3, all diagonal)."""
                j, t = items[i]
                sts, cq = st_tiles.pop(i)
                if t == 0:
                    ctx_tiles[(j, 0)] = psC.tile([65, 512], f32, tag="ctx", name="ctx0")
                    ctx_tiles[(j, 1)] = psC.tile([65, 512], f32, tag="ctx", name="ctx1")
                for h in range(2):
                    pt = ppt.tile([128, 512], bf16, tag="pt", name="pt")
                    nc.scalar.activation(
                        pt[:, cq:512], sts[h][:, cq:512], Act.Exp, scale=0.125
                    )
                    emit_mask(lambda a, b: pt[:, a:b], 0, t, 0, tri)
                    nc.tensor.matmul(
                        ctx_tiles[(j, h)][:],
                        vho16[:, t, h, :],
                        pt[:],
                        start=(t == 0),
                        stop=(t == 3),
                    )
                if t == 3:
                    emit_norm(j)

            def emit_pv(i):
                """fp8 DoubleRow pair path for blocks j>=1."""
                j, t = items[i]
                sts, cq = st_tiles.pop(i)
                c0 = c0_of(j, t)
                p, tt = divmod(t, 2)
                if tt == 0:
                    for h in range(2):
                        pt8_tiles[(p, h)] = ppt8.tile(
                            [128, 2, 512], fp8, tag="pt8", name="pt8"
                        )
                if t == 0:
                    ctx_tiles[(j, 0)] = psC.tile([65, 512], f32, tag="ctx", name="ctx0")
                    ctx_tiles[(j, 1)] = psC.tile([65, 512], f32, tag="ctx", name="ctx1")
                for h in range(2):
                    pt8 = pt8_tiles[(p, h)]
                    if use_act(i, h):
                        nc.scalar.activation(
                            pt8[:, tt, cq:512], sts[h][:, cq:512], Act.Exp,
                            scale=0.125, bias=nbias[:],
                        )
                    else:
                        nc.vector.tensor_scalar(
                            pt8[:, tt, cq:512].bitcast(u8), sts[h][:, cq:512],
                            A8, B8, Alu.mult, Alu.add,
                        )
                    if t - 4 * j >= 0:
                        emit_mask(lambda a, b: pt8[:, tt, a:b], j, t, c0, tri8)
                if tt == 1:
                    g, up = divmod(2 * p, 4)
                    for h in range(2):
                        pt8 = pt8_tiles.pop((p, h))
                        nc.tensor.matmul(
                            ctx_tiles[(j, h)][:, c0:512],
                            vho8[g][:, up : up + 2, h, 0:65],
                            pt8[:, :, c0:512],
                            start=(t == 1),
                            stop=(t == 4 * j + 3),
                            perf_mode=DR,
                        )
                    if t == 4 * j + 3:
                        emit_norm(j)

            # ---------------------------------------------------------------
            items = [(j, t) for j in range(NQB) for t in range(4 * j + 4)]
            emit_proj(0)
            emit_qk(0)
            if len(items) > 1:
                emit_qk(1)
            if NQB > 1:
                emit_proj(1)
            emit_consts()
            for i in range(len(items)):
                if i + 2 < len(items):
                    emit_qk(i + 2)
                if deferred:
                    deferred.pop(0)()
                if items[i][0] == 0:
                    emit_pv_j0(i)
                else:
                    emit_pv(i)
            while deferred:
                deferred.pop(0)()

    nc.compile()
    return nc


def make_in_maps(q, k, v, W_q, W_k, W_v, W_o, b_o, S=4096):
    B = q.shape[0]
    q = np.asarray(q, dtype=np.float32)
    k = np.asarray(k, dtype=np.float32)
    v = np.asarray(v, dtype=np.float32)
    W_q = np.asarray(W_q, dtype=np.float32)
    W_k = np.asarray(W_k, dtype=np.float32)
    W_v = np.asarray(W_v, dtype=np.float32)
    W_o = np.asarray(W_o, dtype=np.float32)
    bf = ml_dtypes.bfloat16
    e4 = ml_dtypes.float8_e4m3

    qT = [np.ascontiguousarray(q[b].T).astype(bf) for b in range(B)]
    kT = [np.ascontiguousarray(k[b].T).astype(bf) for b in range(B)]
    vT = [np.ascontiguousarray(v[b].T).astype(bf) for b in range(B)]

    kk = np.arange(128)[:, None]
    qq = np.arange(128)[None, :]
    trif = (kk <= qq).astype(np.float32)  # [128, 128] local triangle

    in_maps = []
    for c in range(8):
        b, p = divmod(c, 4)
        rows = slice(128 * p, 128 * p + 128)

        def wtile(W):
            wT = W[rows].T.reshape(4, 128, 128).transpose(1, 0, 2)
            return np.ascontiguousarray(wT).astype(bf).reshape(128, 512)
        in_maps.append(
            {
                "qT": qT[b],
                "kT": kT[b],
                "vT": vT[b],
                "wqT": wtile(W_q),
                "wkT": wtile(W_k),
                "wvT": wtile(W_v),
                "woT": np.ascontiguousarray(W_o[:, rows].T).astype(bf),
                "masks": trif.astype(bf),
                "masks8": trif.astype(e4),
            }
        )
    return in_maps


def gather(results, b_o=None, S=4096):
    outT = [r["outT"] for r in results]
    out0 = (outT[0] + outT[1] + outT[2] + outT[3]).T
    out1 = (outT[4] + outT[5] + outT[6] + outT[7]).T
    out = np.stack([out0, out1]).astype(np.float32)
    if b_o is not None:
        out += np.asarray(b_o, dtype=np.float32)
    return out


_nc_cache = {}


def get_nc(S=4096):
    if S not in _nc_cache:
        _nc_cache[S] = build(S)
    return _nc_cache[S]


def kernel(q, k, v, W_q, W_k, W_v, W_o, b_o):
    nc = get_nc(4096)
    in_maps = make_in_maps(q, k, v, W_q, W_k, W_v, W_o, b_o, S=4096)
    res = run_bass_kernel_spmd(nc, in_maps, core_ids=list(range(8)))
    return gather(res.results, b_o=b_o)


# revision 59
# speedup vs baseline: 1.1476x; 1.0097x over previous
"""Multi-head causal attention (B=2, S=4096, D=512, H=8) on 8 NeuronCores.

Sharding: batch x head-pair. Core c handles batch b = c//4 and heads
{2*(c%4), 2*(c%4)+1}. Each core computes its 2 heads' projections, causal
flash attention, and a partial out-projection (its heads' rank-128 slice of
W_o). Partials of the 4 cores sharing a batch are summed on the host during
the gather (tensor-parallel all-reduce); b_o is added on the host too.

Device design:
  - scores computed transposed: S.T [k, q] tiles so PV needs no transposes;
    per-q row-sums come from an ones-column appended to V (PV matmul M=65)
  - softmax without a running max; for blocks j>=1 the exp is biased by -3
    (exp(s/8 - 3)) so probabilities fit fp8e4 range; the bias cancels in the
    normalization. Block j=0 (first 512 q) keeps a bf16 path since its early
    rows can have tiny row maxima that would flush to zero in fp8.
  - exp is split per (k-tile, head) across two engines: roughly half the
    head-halves use the Scalar ACT table exp, the rest compute exp on
    VectorE as a Schraudolph-style affine in fp8-bitpattern space (uint8
    saturating convert of s*A8+B8 == e4m3 bits of exp(s/8-3), within ~6%);
    each tile's two exps run concurrently on two engines, and the per-head
    1-bank st slots (psA bufs=4) give four parallel exp->QK recycle chains
  - PV for j>=1 runs as fp8 DoubleRow matmuls over k-tile PAIRS:
    lhsT = vho8[128, 2, 65] (stride-160 pair slices), rhs = pt8[128, 2, 512]
  - V is projected directly into transposed [kpos, d] layout (lhsT = the
    x chunk, rhs = W_v slice), so no PE transposes / identity matrix
  - causal masking decomposed: the triangular boundary is only a [128, 128]
    subtile (one shared local-triangle multiply on VectorE); fully-masked
    128-col stripes are memset to 0 on GpSimd; fully-masked column PAIRS
    are skipped in QK/exp/PV
  - the normalization + out-projection of each block is emitted DEFERRED,
    one stage per subsequent item, so its cross-engine chain (scalar ones-
    row copy -> vector recip -> gpsimd broadcast -> vector mul -> PE
    out-proj) never blocks the next block's per-tile work in any queue
  - projection work is spread across the item loop (one part per QK)
"""

import numpy as np
import ml_dtypes

import concourse.bass as bass
import concourse.bacc as bacc
import concourse.mybir as mybir
import concourse.tile as tile
from concourse.bass_utils import run_bass_kernel_spmd

D = 512
BSHIFT = 3.0  # exp bias for fp8 path: pt = exp(s/8 - BSHIFT)
LOG2E = float(np.log2(np.e))
A8 = 0.125 * LOG2E * 8.0
B8 = 8.0 * (7.0 - BSHIFT * LOG2E - 0.0436)

f32 = mybir.dt.float32
bf16 = mybir.dt.bfloat16
fp8 = mybir.dt.float8e4
u8 = mybir.dt.uint8
ts = bass.ts
Act = mybir.ActivationFunctionType
Alu = mybir.AluOpType
DR = mybir.MatmulPerfMode.DoubleRow


def build(S=4096):
    NQB = S // 512  # q-blocks / s-blocks / k-groups

    nc = bacc.Bacc("TRN2", target_bir_lowering=False, debug=False, num_devices=8)

    qT_d = nc.dram_tensor("qT", [D, S], bf16, kind="ExternalInput").ap()
    kT_d = nc.dram_tensor("kT", [D, S], bf16, kind="ExternalInput").ap()
    vT_d = nc.dram_tensor("vT", [D, S], bf16, kind="ExternalInput").ap()
    wqT_d = nc.dram_tensor("wqT", [128, D], bf16, kind="ExternalInput").ap()
    wkT_d = nc.dram_tensor("wkT", [128, D], bf16, kind="ExternalInput").ap()
    wvT_d = nc.dram_tensor("wvT", [128, D], bf16, kind="ExternalInput").ap()
    woT_d = nc.dram_tensor("woT", [128, D], bf16, kind="ExternalInput").ap()
    masks_d = nc.dram_tensor("masks", [128, 128], bf16, kind="ExternalInput").ap()
    masks8_d = nc.dram_tensor("masks8", [128, 128], fp8, kind="ExternalInput").ap()
    outT_d = nc.dram_tensor("outT", [D, S], f32, kind="ExternalOutput").ap()

    with tile.TileContext(nc) as tc:
        with (
            tc.tile_pool(name="const", bufs=1) as pc,
            tc.tile_pool(name="persist", bufs=1) as pp,
            tc.tile_pool(name="chunk", bufs=10) as pch,
            tc.tile_pool(name="pt", bufs=4) as ppt,
            tc.tile_pool(name="pt8", bufs=8) as ppt8,
            tc.tile_pool(name="small", bufs=3) as psm,
            tc.tile_pool(name="ostage", bufs=2) as pos,
            tc.tile_pool(name="psP", bufs=2, space="PSUM") as psP,
            tc.tile_pool(name="psA", bufs=4, space="PSUM") as psA,
            tc.tile_pool(name="psC", bufs=2, space="PSUM") as psC,
        ):
            tri = pc.tile([128, 128], bf16, tag="tri")
            tri8 = pc.tile([128, 128], fp8, tag="tri8")
            wq = pc.tile([128, 4, 128], bf16, tag="wq")
            wk = pc.tile([128, 4, 128], bf16, tag="wk")
            wv = pc.tile([128, 4, 128], bf16, tag="wv")
            wo = pc.tile([128, D], bf16, tag="wo")
            nbias = pc.tile([128, 1], f32, tag="nbias")
            nc.gpsimd.memset(nbias[:], -BSHIFT)
            nc.sync.dma_start(tri[:], masks_d)
            nc.sync.dma_start(tri8[:], masks8_d)
            nc.sync.dma_start(wk[:], wkT_d.rearrange("p (e m) -> p e m", e=4))
            nc.sync.dma_start(wq[:], wqT_d.rearrange("p (e m) -> p e m", e=4))
            nc.sync.dma_start(wv[:], wvT_d.rearrange("p (e m) -> p e m", e=4))

            def emit_consts():
                nc.sync.dma_start(wo[:], woT_d)

            khT = [pp.tile([128, 512], bf16, tag=f"khT{g}", name=f"khT{g}") for g in range(NQB)]
            qhT = [pp.tile([128, 512], bf16, tag=f"qhT{g}", name=f"qhT{g}") for g in range(NQB)]
            ctxT = [pp.tile([128, 512], bf16, tag=f"ctxT{g}", name=f"ctxT{g}") for g in range(NQB)]
            # [128 kpos, 4 u, 2 heads, 80]: fp8 V with ones col 64; pair slice
            # [:, u:u+2, h, 0:65] has dim1 stride 160 (%16==0 for dual-fp8 LDW)
            vho8 = [pp.tile([128, 4, 2, 80], fp8, tag=f"vho8_{g}", name=f"vho8_{g}") for g in range(NQB)]
            vho16 = pp.tile([128, 4, 2, 65], bf16, tag="vho16", name="vho16")
            nc.gpsimd.memset(vho16[:, :, :, 64:65], 1.0)
            for g in range(NQB):
                nc.gpsimd.memset(vho8[g][:, :, :, 64:65], 1.0)

            # ---------------------------------------------------------------

            def dma_proj(j, src_d):
                """Prefetch all 4 input chunks of a projection in ONE strided
                DMA ([512, 512] dram block -> [128, 4, 512] tile) so neither
                DMA latency nor Sync-queue issue cost gates the matmuls."""
                ch = pch.tile([128, 4, 512], bf16, tag="chunk", name="ch")
                nc.sync.dma_start(
                    ch[:], src_d[:, ts(j, 512)].rearrange("(e p) c -> p e c", e=4)
                )
                return ch

            def emit_proj_kq(j, ch, w, dst):
                slot = psP.tile([128, 512], f32, tag="pp", name="pp")
                for e in range(4):
                    nc.tensor.matmul(
                        slot[:], w[:, e, :], ch[:, e, :], start=(e == 0), stop=(e == 3)
                    )
                nc.scalar.copy(dst[j][:], slot[:])

            def emit_proj_v(j, ch, us):
                # v projected straight into [kpos, d2] layout for subchunks us
                slot = psP.tile([128, 512], f32, tag="pp", name="slotv")
                for uu in us:
                    for e in range(4):
                        nc.tensor.matmul(
                            slot[:, ts(uu, 128)],
                            ch[:, e, ts(uu, 128)],
                            wv[:, e, :],
                            start=(e == 0),
                            stop=(e == 3),
                        )
                    src = slot[:, ts(uu, 128)].rearrange("p (h d) -> p h d", h=2)
                    nc.vector.tensor_copy(vho8[j][:, uu, :, 0:64], src)
                    if j == 0:
                        nc.vector.tensor_copy(vho16[:, uu, :, 0:64], src)

            pending_parts = []

            def queue_proj(j):
                chk = dma_proj(j, kT_d)
                chq = dma_proj(j, qT_d)
                chv = dma_proj(j, vT_d)
                pending_parts.append(lambda: emit_proj_kq(j, chk, wk, khT))
                pending_parts.append(lambda: emit_proj_kq(j, chq, wq, qhT))
                pending_parts.append(lambda: emit_proj_v(j, chv, (0, 1)))
                pending_parts.append(lambda: emit_proj_v(j, chv, (2, 3)))

            def emit_proj(j):
                emit_proj_kq(j, dma_proj(j, kT_d), wk, khT)
                emit_proj_kq(j, dma_proj(j, qT_d), wq, qhT)
                chv = dma_proj(j, vT_d)
                emit_proj_v(j, chv, (0, 1))
                emit_proj_v(j, chv, (2, 3))

            deferred = []

            ob_tiles = {}

            def emit_outproj_ot(j, ot):
                op = psP.tile([128, 512], f32, tag="pp", name="op")
                nc.tensor.matmul(
                    op[:], wo[:, ts(ot, 128)], ctxT[j][:], start=True, stop=True
                )
                if ot == 0:
                    ob_tiles[j] = pos.tile([128, 4, 512], f32, tag="ob", name="ob")
                ob = ob_tiles[j]
                if ot % 2 == 0:
                    nc.vector.tensor_copy(ob[:, ot, :], op[:])
                else:
                    nc.scalar.copy(ob[:, ot, :], op[:])
                # per-ot DMA, dispatched from the (idle) gpsimd queue right
                # after its copy: keeps outT traffic off the Sync queue AND
                # avoids one huge serialized transfer blocking the gpsimd
                # queue (whose next op is the following block's broadcast)
                nc.gpsimd.dma_start(outT_d[ts(ot, 128), ts(j, 512)], ob[:, ot, :])
                if ot == 3:
                    ob_tiles.pop(j)

            ctx_tiles = {}
            st_tiles = {}
            pt8_tiles = {}
            norm_state = {}

            def emit_norm(j):
                """Queue the normalization + out-projection stages for block j;
                they are drained one per subsequent item so they never block
                the next block's per-tile work."""
                ctxs = [ctx_tiles.pop((j, h)) for h in range(2)]

                # stage 1 immediately: ones-row copy + reciprocal
                lrow = psm.tile([1, 2, 512], f32, tag="lrow", name="lrow", bufs=2)
                for h in range(2):
                    nc.scalar.copy(lrow[:, h, :], ctxs[h][64:65, :])
                r = psm.tile([1, 2, 512], f32, tag="r", name="r", bufs=2)
                nc.vector.reciprocal_approx_fast(
                    r[:].rearrange("p a b -> p (a b)"),
                    lrow[:].rearrange("p a b -> p (a b)"),
                )
                norm_state[j] = r

                def stage2():
                    r = norm_state.pop(j)
                    rbc = psm.tile([64, 2, 512], f32, tag="rbc", name="rbc", bufs=2)
                    nc.gpsimd.partition_broadcast(
                        rbc[:].rearrange("p a b -> p (a b)"),
                        r[:].rearrange("p a b -> p (a b)"),
                    )
                    for h in range(2):
                        nc.vector.tensor_mul(
                            ctxT[j][64 * h : 64 * h + 64, :],
                            ctxs[h][0:64, :],
                            rbc[:, h, :],
                        )

                deferred.append(stage2)
                for ot in range(4):
                    deferred.append(lambda ot=ot: emit_outproj_ot(j, ot))

            def c0_of(j, t):
                # pair-granular masked-column start (PV + memset base)
                u = t - 4 * j
                if j >= 1 and u >= 2:
                    return 256
                return 0

            def c0_qk(j, t):
                # tile-granular start for QK + exp: columns below 128*u are
                # fully masked; the gpless memset zeroes [c0_of, 128u) of pt
                # so PV still reads valid zeros there
                u = t - 4 * j
                if u >= 1:
                    return 128 * u
                return 0

            def emit_qk(i):
                j, t = items[i]
                if t == 0 and j + 2 < NQB and j + 2 >= 2:
                    queue_proj(j + 2)
                # per-head 1-bank st slots; 5 slots so the second head's
                # slot is almost always free and the QK pair launches
                # concurrently on the PE
                sts = [
                    psA.tile([128, 512], f32, tag="st", name=f"st{h}")
                    for h in range(2)
                ]
                c0 = c0_qk(j, t)
                for h in range(2):
                    nc.tensor.matmul(
                        sts[h][:, c0:512],
                        khT[t // 4][64 * h : 64 * h + 64, ts(t % 4, 128)],
                        qhT[j][64 * h : 64 * h + 64, c0:512],
                        start=True, stop=True, tile_position=(64 * h, 0),
                    )
                st_tiles[i] = (sts, c0)
                # proj work AFTER the QK pair: it fills the PE's exp-wait
                # time instead of sitting between the freed st slot and the
                # next QK in the queue
                if pending_parts:
                    pending_parts.pop(0)()

            def emit_mask(sl, j, t, c0, trit):
                """Causal masking for diagonal tile (j, t), one head: memset
                the fully masked 128-col stripes, multiply the local triangle.
                sl(a, b) -> the [128, b-a] AP of this head's columns a:b."""
                u = t - 4 * j
                if 128 * u > c0:
                    nc.vector.memset(sl(c0, 128 * u), 0.0)
                nc.vector.tensor_mul(
                    sl(128 * u, 128 * u + 128),
                    sl(128 * u, 128 * u + 128),
                    trit[:],
                )

            def use_act(i, h):
                # chains (2i+h)%4 in {0, 3} -> ACT, {1, 2} -> DVE, plus a few
                # DVE-chain tiles shifted to ACT for engine balance
                c = (2 * i + h) % 4
                if c in (0, 3):
                    return True
                return i % 8 == 1 and h == 0

            def emit_pv_j0(i):
                """bf16 per-tile path for block j=0 (t = 0..3, all diagonal)."""
                j, t = items[i]
                sts, cq = st_tiles.pop(i)
                if t == 0:
                    ctx_tiles[(j, 0)] = psC.tile([65, 512], f32, tag="ctx", name="ctx0")
                    ctx_tiles[(j, 1)] = psC.tile([65, 512], f32, tag="ctx", name="ctx1")
                for h in range(2):
                    pt = ppt.tile([128, 512], bf16, tag="pt", name="pt")
                    nc.scalar.activation(
                        pt[:, cq:512], sts[h][:, cq:512], Act.Exp, scale=0.125
                    )
                    emit_mask(lambda a, b: pt[:, a:b], 0, t, 0, tri)
                    nc.tensor.matmul(
                        ctx_tiles[(j, h)][:],
                        vho16[:, t, h, :],
                        pt[:],
                        start=(t == 0),
                        stop=(t == 3),
                    )
                if t == 3:
                    emit_norm(j)

            def emit_pv(i):
                """fp8 DoubleRow pair path for blocks j>=1."""
                j, t = items[i]
                sts, cq = st_tiles.pop(i)
                c0 = c0_of(j, t)
                p, tt = divmod(t, 2)
                if tt == 0:
                    for h in range(2):
                        pt8_tiles[(p, h)] = ppt8.tile(
                            [128, 2, 512], fp8, tag="pt8", name="pt8"
                        )
                if t == 0:
                    ctx_tiles[(j, 0)] = psC.tile([65, 512], f32, tag="ctx", name="ctx0")
                    ctx_tiles[(j, 1)] = psC.tile([65, 512], f32, tag="ctx", name="ctx1")
                for h in range(2):
                    pt8 = pt8_tiles[(p, h)]
                    if use_act(i, h):
                        nc.scalar.activation(
                            pt8[:, tt, cq:512], sts[h][:, cq:512], Act.Exp,
                            scale=0.125, bias=nbias[:],
                        )
                    else:
                        nc.vector.tensor_scalar(
                            pt8[:, tt, cq:512].bitcast(u8), sts[h][:, cq:512],
                            A8, B8, Alu.mult, Alu.add,
                        )
                    if t - 4 * j >= 0:
                        emit_mask(lambda a, b: pt8[:, tt, a:b], j, t, c0, tri8)
                if tt == 1:
                    g, up = divmod(2 * p, 4)
                    for h in range(2):
                        pt8 = pt8_tiles.pop((p, h))
                        nc.tensor.matmul(
                            ctx_tiles[(j, h)][:, c0:512],
                            vho8[g][:, up : up + 2, h, 0:65],
                            pt8[:, :, c0:512],
                            start=(t == 1),
                            stop=(t == 4 * j + 3),
                            perf_mode=DR,
                        )
                    if t == 4 * j + 3:
                        emit_norm(j)

            # ---------------------------------------------------------------
            items = [(j, t) for j in range(NQB) for t in range(4 * j + 4)]
            emit_proj(0)
            emit_qk(0)
            if len(items) > 1:
                emit_qk(1)
            if NQB > 1:
                emit_proj(1)
            emit_consts()
            for i in range(len(items)):
                if i + 2 < len(items):
                    emit_qk(i + 2)
                if deferred:
                    deferred.pop(0)()
                if items[i][0] == 0:
                    emit_pv_j0(i)
                else:
                    emit_pv(i)
            while deferred:
                deferred.pop(0)()

    nc.compile()
    return nc


def make_in_maps(q, k, v, W_q, W_k, W_v, W_o, b_o, S=4096):
    B = q.shape[0]
    q = np.asarray(q, dtype=np.float32)
    k = np.asarray(k, dtype=np.float32)
    v = np.asarray(v, dtype=np.float32)
    W_q = np.asarray(W_q, dtype=np.float32)
    W_k = np.asarray(W_k, dtype=np.float32)
    W_v = np.asarray(W_v, dtype=np.float32)
    W_o = np.asarray(W_o, dtype=np.float32)
    bf = ml_dtypes.bfloat16
    e4 = ml_dtypes.float8_e4m3

    qT = [np.ascontiguousarray(q[b].T).astype(bf) for b in range(B)]
    kT = [np.ascontiguousarray(k[b].T).astype(bf) for b in range(B)]
    vT = [np.ascontiguousarray(v[b].T).astype(bf) for b in range(B)]

    kk = np.arange(128)[:, None]
    qq = np.arange(128)[None, :]
    trif = (kk <= qq).astype(np.float32)  # [128, 128] local triangle

    in_maps = []
    for c in range(8):
        b, p = divmod(c, 4)
        rows = slice(128 * p, 128 * p + 128)

        def wtile(W):
            wT = W[rows].T.reshape(4, 128, 128).transpose(1, 0, 2)
            return np.ascontiguousarray(wT).astype(bf).reshape(128, 512)
        in_maps.append(
            {
                "qT": qT[b],
                "kT": kT[b],
                "vT": vT[b],
                "wqT": wtile(W_q),
                "wkT": wtile(W_k),
                "wvT": wtile(W_v),
                "woT": np.ascontiguousarray(W_o[:, rows].T).astype(bf),
                "masks": trif.astype(bf),
                "masks8": trif.astype(e4),
            }
        )
    return in_maps


def gather(results, b_o=None, S=4096):
    outT = [r["outT"] for r in results]
    out0 = (outT[0] + outT[1] + outT[2] + outT[3]).T
    out1 = (outT[4] + outT[5] + outT[6] + outT[7]).T
    out = np.stack([out0, out1]).astype(np.float32)
    if b_o is not None:
        out += np.asarray(b_o, dtype=np.float32)
    return out


_nc_cache = {}


def get_nc(S=4096):
    if S not in _nc_cache:
        _nc_cache[S] = build(S)
    return _nc_cache[S]


def kernel(q, k, v, W_q, W_k, W_v, W_o, b_o):
    nc = get_nc(4096)
    in_maps = make_in_maps(q, k, v, W_q, W_k, W_v, W_o, b_o, S=4096)
    res = run_bass_kernel_spmd(nc, in_maps, core_ids=list(range(8)))
    return gather(res.results, b_o=b_o)
